# revision 37
# baseline (speedup 1.0000x reference)
"""Local/global multihead attention on 8 NeuronCores (Trainium2, Bass/Tile).

Sharding: core c = b*2 + hg  (b = batch 0..3, hg = head-group 0/1, 8 heads each).

v3 design (vs v2 baseline at 270us):
- slot0's dense 2048-key path is split into a normal banded stage plus a
  "far-field" stage covering the band-window complement at block granularity.
  The far field needs no (key,query) mask: on hg0 (true global head) it is
  unmasked; on hg1 it is killed by a per-key additive bias (-60) folded into
  the exp activation (exp(scale*s - 60) ~= 0).  This removes the 8MB gmask
  DMA and all dense-path mask multiplies.  slot0's banded stage uses a
  per-core window mask input (hg0: ones -> keeps band+corners; hg1: band).
- PE row-tile half-alternation: consecutive K=64 QK matmuls alternate
  between array rows 0-63 and 64-127 (measured 3x on a microbench: LDWEIGHTS
  pulls ahead + both halves stream concurrently).  Far-field units alternate
  halves via a duplicated copy of slot0's q/k in partitions 64-127 of a 5th
  pair tile; banded stages alternate naturally by slot parity and are
  emitted chunk-interleaved in parity pairs.
- AV matmuls are split along the key contraction into rows 0-63 / 64-127
  (tiles (0,0)/(64,0)), accumulated in two PSUM banks, recombined by one
  fused vector op during the divide step.
- divide: reciprocal reads the summed avs tile directly (no den copy);
  out-projection PSUM->SBUF copies moved to gpsimd.

All matmul operands bf16; PSUM fp32.  Host sums the two head-group partials
per batch and adds bo + bv @ wo.T (softmax rows sum to 1; bk cancels in
softmax).
"""
import numpy as np
import ml_dtypes

E, H, D, LK = 1024, 16, 64, 128
SCALE = D ** -0.5
B, N = 4, 2048
FG = 512          # features per head-group (8 heads * 64)
NCORES = 8

# narrowed tq windows per dj variant (delta = (dj-1)*128)
WIN = [(0, 128), (0, 256), (0, 384), (128, 512), (256, 512), (384, 512)]
# packed column offsets of each dj window inside the [128,1536] at tile
POS = [0, 128, 384, 768, 1152, 1408]

# far-field units per s-block: (jc, q0, q1) — q range local to the s-block,
# covering the complement of the band windows at block granularity.
FAR = []
for _s in range(4):
    _near = {}
    for _dj in range(6):
        _jc = _s * 4 - 1 + _dj
        if 0 <= _jc <= 15:
            _near[_jc] = _dj
    _units = []
    for _jc in range(16):
        if _jc in _near:
            _c0, _c1 = WIN[_near[_jc]]
            _q0, _q1 = (0, _c0) if _c0 > 0 else (_c1, 512)
        else:
            _q0, _q1 = 0, 512
        if _q1 > _q0:
            _units.append((_jc, _q0, _q1))
    # full-width units first: the first matmul of the psum accumulation
    # group then covers the whole bank, so later partial-width writes land
    # on uniformly-cleared bytes (keeps the interp's pending-zero model
    # happy; hardware per-element has_written is fine either way)
    _units.sort(key=lambda u: u[2] - u[1], reverse=True)
    FAR.append(_units)


def _av_order(djs):
    """Order band-AV dj emissions so each psum write region is entirely
    inside or entirely outside the already-written byte range."""
    order = []
    cleared = set()
    rem = list(djs)
    while rem:
        for dj in rem:
            reg = set(range(*WIN[dj]))
            if not order or reg <= cleared or not (reg & cleared):
                order.append(dj)
                rem.remove(dj)
                cleared |= reg
                break
        else:
            raise AssertionError(f"no valid av order for {djs}")
    return order

_cache = {}
DEBUG_DUMP = False
FAR_ALT = True


def _bf16(a):
    return np.ascontiguousarray(a.astype(ml_dtypes.bfloat16))


def _build():
    import concourse.bacc as bacc
    import concourse.tile as tile
    import concourse.mybir as mybir
    from concourse.bass import ts

    dt = mybir.dt
    AF = mybir.ActivationFunctionType
    ALU = mybir.AluOpType

    nc = bacc.Bacc("TRN2", target_bir_lowering=False, debug=False,
                   num_devices=NCORES)

    xT = nc.dram_tensor("xT", [E, N], dt.bfloat16, kind="ExternalInput")
    wqT = nc.dram_tensor("wqT", [E, FG], dt.bfloat16, kind="ExternalInput")
    wkT = nc.dram_tensor("wkT", [E, FG], dt.bfloat16, kind="ExternalInput")
    wvT = nc.dram_tensor("wvT", [E, FG], dt.bfloat16, kind="ExternalInput")
    woT = nc.dram_tensor("woT", [FG, E], dt.bfloat16, kind="ExternalInput")
    bqc = nc.dram_tensor("bqc", [128, 4], dt.float32, kind="ExternalInput")
    lmask = nc.dram_tensor("lmask", [128, 1536], dt.bfloat16, kind="ExternalInput")
    lmask0 = nc.dram_tensor("lmask0", [128, 1536], dt.bfloat16, kind="ExternalInput")
    fbias = nc.dram_tensor("fbias", [128, 1], dt.float32, kind="ExternalInput")
    out = nc.dram_tensor("out", [N, E], dt.bfloat16, kind="ExternalOutput")
    if DEBUG_DUMP:
        dbg_q = nc.dram_tensor("dbg_q", [128, N], dt.bfloat16, kind="ExternalOutput")
        dbg_k4 = nc.dram_tensor("dbg_k4", [64, N], dt.bfloat16, kind="ExternalOutput")
        dbg_v = nc.dram_tensor("dbg_v", [128, 65], dt.bfloat16, kind="ExternalOutput")
        dbg_o = nc.dram_tensor("dbg_o", [4 * 128, N], dt.bfloat16, kind="ExternalOutput")
        dbg_avs = nc.dram_tensor("dbg_avs", [65, 512], dt.float32, kind="ExternalOutput")
        dbg_rec = nc.dram_tensor("dbg_rec", [1, 512], dt.float32, kind="ExternalOutput")
        dbg_at = nc.dram_tensor("dbg_at", [128, 1408], dt.bfloat16, kind="ExternalOutput")
        dbg_ft = nc.dram_tensor("dbg_ft", [128, 512], dt.bfloat16, kind="ExternalOutput")

    with tile.TileContext(nc) as tc:
        with (
            tc.tile_pool(name="wts", bufs=1) as wts,
            tc.tile_pool(name="xp", bufs=1) as xp,
            tc.tile_pool(name="qkv", bufs=1) as qkv,
            tc.tile_pool(name="att", bufs=3) as att,
            tc.tile_pool(name="fat", bufs=2) as fatp,
            tc.tile_pool(name="small", bufs=4) as small,
            tc.tile_pool(name="ps", bufs=1, space="PSUM") as psp,
        ):
            # ---- load weights/x/masks (ordered so qk_proj(0) starts ASAP) ----
            xT_t = [xp.tile([128, N], dt.bfloat16, name=f"xT{i}", tag=f"xT{i}") for i in range(8)]
            wq_t = [wts.tile([128, FG], dt.bfloat16, name=f"wq{i}", tag=f"wq{i}") for i in range(8)]
            wk_t = [wts.tile([128, FG], dt.bfloat16, name=f"wk{i}", tag=f"wk{i}") for i in range(8)]
            wv_t = [wts.tile([128, FG], dt.bfloat16, name=f"wv{i}", tag=f"wv{i}") for i in range(8)]
            bq_t = small.tile([128, 4], dt.float32, name="bq", tag="bq")
            fb_t = small.tile([128, 1], dt.float32, name="fb", tag="fb")
            nc.sync.dma_start(bq_t[:], bqc[:, :])
            nc.sync.dma_start(fb_t[:], fbias[:, :])
            for ec in range(8):
                nc.sync.dma_start(xT_t[ec][:], xT[ts(ec, 128), :])
                nc.sync.dma_start(wq_t[ec][:], wqT[ts(ec, 128), :])
                nc.sync.dma_start(wk_t[ec][:], wkT[ts(ec, 128), :])
            for ec in range(8):
                nc.sync.dma_start(wv_t[ec][:], wvT[ts(ec, 128), :])
            lm_t = wts.tile([128, 1536], dt.bfloat16, name="lm", tag="lm")
            nc.sync.dma_start(lm_t[:], lmask[:, :])
            lm0_t = wts.tile([128, 1536], dt.bfloat16, name="lm0", tag="lm0")
            nc.sync.dma_start(lm0_t[:], lmask0[:, :])
            wo_t = [wts.tile([128, E], dt.bfloat16, name=f"wo{i}", tag=f"wo{i}") for i in range(4)]
            for fc in range(4):
                nc.sync.dma_start(wo_t[fc][:], woT[ts(fc, 128), :])

            qT_sb = [qkv.tile([128, N], dt.bfloat16, name=f"qT{i}", tag=f"qT{i}") for i in range(4)]
            kT_sb = [qkv.tile([128, N], dt.bfloat16, name=f"kT{i}", tag=f"kT{i}") for i in range(4)]
            qT4 = qkv.tile([128, N], dt.bfloat16, name="qT4", tag="qT4")
            kT4 = qkv.tile([128, N], dt.bfloat16, name="kT4", tag="kT4")
            v_sb = [qkv.tile([128, 8 * 72], dt.bfloat16, name=f"v{i}", tag=f"v{i}") for i in range(16)]
            outTn = [qkv.tile([128, N], dt.bfloat16, name=f"outTn{i}", tag=f"outTn{i}") for i in range(4)]

            ring_state = [0]
            dbg_tiles = {}

            def ring_tile():
                t = psp.tile([128, 512], dt.float32, name="rg",
                             tag=f"rg{ring_state[0] % 2}")
                ring_state[0] += 1
                return t

            def qk_proj_emit(fc, dst_is_q, ec, accs):
                """one ec chunk (4 tcn matmuls) of q or k projection for fc."""
                w_t = wq_t if dst_is_q else wk_t
                for tcn in range(4):
                    nc.tensor.matmul(
                        accs[tcn][:], w_t[ec][:, ts(fc, 128)],
                        xT_t[ec][:, ts(tcn, 512)],
                        start=(ec == 0), stop=(ec == 7))
                if ec == 7:
                    dst = qT_sb if dst_is_q else kT_sb
                    for tcn in range(4):
                        if dst_is_q:
                            nc.scalar.activation(
                                dst[fc][:, ts(tcn, 512)], accs[tcn][:],
                                AF.Identity, bias=bq_t[:, fc:fc + 1])
                        else:
                            nc.vector.tensor_copy(dst[fc][:, ts(tcn, 512)],
                                                  accs[tcn][:])

            def make_accs():
                return [psp.tile([128, 512], dt.float32, name=f"acc{t}",
                                 tag=f"acc{t}") for t in range(4)]

            def v_proj(tcn):
                """v chunk tcn: natural layout, per-head 72-col strided + ones col."""
                ps = ring_tile()
                for ec in range(8):
                    nc.tensor.matmul(ps[:], xT_t[ec][:, ts(tcn, 128)], wv_t[ec][:],
                                     start=(ec == 0), stop=(ec == 7))
                src = ps[:].rearrange("p (h d) -> p h d", h=8)
                dst = v_sb[tcn][:].rearrange("p (h d) -> p h d", h=8)[:, :, 0:64]
                nc.vector.tensor_copy(dst, src)
                ones = v_sb[tcn][:].rearrange("p (h d) -> p h d", h=8)[:, :, 64:65]
                nc.vector.memset(ones, 1.0)

            def head_rows(t, h):
                r0 = (h % 2) * 64
                return t[h // 2][r0:r0 + 64, :]

            def divide(slot, s, avl, avh=None):
                """outTn rows for (slot, s) = num / den from the av psum
                tile(s).  avh=None: single psum tile (read directly).  With
                avh: split-K pair, combined into an SBUF tile first (gpsimd
                can't read PSUM; DVE ops may read at most one PSUM input)."""
                if avh is not None:
                    avs = small.tile([65, 512], dt.float32, name="avs",
                                     tag="avs", bufs=2)
                    nc.vector.tensor_copy(avs[:], avl[0:65, :])
                    nc.vector.tensor_add(avs[:], avs[:], avh[0:65, :])
                    # custom-DVE recip needs a partition-0-based input AP
                    dent = small.tile([1, 512], dt.float32, name="den", tag="den")
                    nc.vector.tensor_copy(dent[:], avs[64:65, :])
                    den = dent[:]
                    dbg_tiles["avs"] = avs
                else:
                    # custom-DVE recip can't read PSUM on HW: den -> SBUF first
                    avs = avl
                    dent = small.tile([1, 512], dt.float32, name="den", tag="den")
                    nc.vector.tensor_copy(dent[:], avl[64:65, :])
                    den = dent[:]
                rec = small.tile([1, 512], dt.float32, name="rec", tag="rec")
                nc.vector.reciprocal_approx_fast(out=rec[:], in_=den)
                if avh is not None:
                    dbg_tiles["rec"] = rec
                rec64 = small.tile([64, 512], dt.float32, name="rec64", tag="rec64")
                nc.gpsimd.partition_broadcast(rec64[:], rec[:])
                nc.vector.tensor_mul(head_rows(outTn, slot)[:, ts(s, 512)],
                                     avs[0:64, :], rec64[:])

            # -------- banded stage pieces (chunked so twins can interleave) --
            def band_prep(slot, s, use_ring):
                if slot == 0:
                    qh = qT_sb[0][0:64, :]
                    kh = kT_sb[0][0:64, :]
                else:
                    qh = head_rows(qT_sb, slot)
                    kh = head_rows(kT_sb, slot)
                djs = [dj for dj in range(6) if 0 <= s * 4 - 1 + dj <= 15]
                lo_at = min(POS[d] for d in djs)
                hi_at = max(POS[d] + WIN[d][1] - WIN[d][0] for d in djs)
                at = att.tile([128, 1536], dt.bfloat16, name="at", tag="at",
                              bufs=6)
                return {"slot": slot, "s": s, "qh": qh, "kh": kh, "djs": djs,
                        "odjs": _av_order(djs),
                        "lo": lo_at, "hi": hi_at, "at": at, "ring": use_ring}

            def band_chunk(st, ci):
                s, djs = st["s"], st["djs"]
                c_lo, c_hi = 512 * ci, 512 * ci + 512
                lo, hi = max(st["lo"], c_lo), min(st["hi"], c_hi)
                if lo >= hi:
                    return
                if st["ring"]:
                    ps = ring_tile()
                else:
                    ps = psp.tile([128, 512], dt.float32, name="bps",
                                  tag=f"acc{(0 if st['slot'] % 2 == 0 else 2) + ci % 2}")
                for dj in djs:
                    jc = s * 4 - 1 + dj
                    w0 = POS[dj]
                    w1 = w0 + WIN[dj][1] - WIN[dj][0]
                    a0, a1 = max(w0, c_lo), min(w1, c_hi)
                    if a0 >= a1:
                        continue
                    q0 = s * 512 + WIN[dj][0] + (a0 - w0)
                    nc.tensor.matmul(
                        ps[:, a0 - c_lo:a1 - c_lo], st["kh"][:, ts(jc, 128)],
                        st["qh"][:, q0:q0 + (a1 - a0)],
                        start=True, stop=True, skip_group_check=True)
                nc.scalar.activation(st["at"][:, lo:hi],
                                     ps[:, lo - c_lo:hi - c_lo],
                                     AF.Exp, scale=float(SCALE))

            def band_mask(st):
                lmt = lm0_t if st["slot"] == 0 else lm_t
                nc.vector.tensor_mul(st["at"][:, st["lo"]:st["hi"]],
                                     st["at"][:, st["lo"]:st["hi"]],
                                     lmt[:, st["lo"]:st["hi"]])

            def band_av(st, avl, avh, cont):
                """split-K AV; accumulates into avl/avh psum tiles."""
                slot, s, odjs, at = st["slot"], st["s"], st["odjs"], st["at"]
                vcol = slot * 72
                for i, dj in enumerate(odjs):
                    jc = s * 4 - 1 + dj
                    c0, c1 = WIN[dj]
                    w = c1 - c0
                    for lohi, avt in ((0, avl), (1, avh)):
                        r0 = lohi * 64
                        nc.tensor.matmul(
                            avt[0:65, c0:c1],
                            v_sb[jc][r0:r0 + 64, vcol:vcol + 65],
                            at[r0:r0 + 64, POS[dj]:POS[dj] + w],
                            start=(i == 0 and not cont),
                            stop=(i == len(odjs) - 1),
                            skip_group_check=True)

            # ================== emission ==================
            # qk projection for pair tile 0 (slots 0,1) upfront
            accs = make_accs()
            for ec in range(8):
                qk_proj_emit(0, True, ec, accs)
            accs = make_accs()
            for ec in range(8):
                qk_proj_emit(0, False, ec, accs)

            # duplicate slot0 q/k into partitions 64-127 of the spare pair tile
            nc.sync.dma_start(qT4[64:128, :], qT_sb[0][0:64, :])
            nc.sync.dma_start(kT4[64:128, :], kT_sb[0][0:64, :])

            # remaining qk projections, emitted chunkwise between far units
            proj_state = {"accs": None, "queue": [],
                          "list": [(fc, dq) for fc in (1, 2, 3)
                                   for dq in (True, False)],
                          "li": 0}

            def emit_next_proj_chunk():
                st = proj_state
                if not st["queue"]:
                    if st["li"] >= len(st["list"]):
                        return
                    fc, dq = st["list"][st["li"]]
                    st["li"] += 1
                    st["accs"] = make_accs()
                    st["queue"] = [(fc, dq, ec) for ec in range(8)]
                fc, dq, ec = st["queue"].pop(0)
                qk_proj_emit(fc, dq, ec, st["accs"])

            def far_av(savl, savh, jc, q0, q1, ft, first):
                w = q1 - q0
                for lohi, avt in ((0, savl), (1, savh)):
                    r0 = lohi * 64
                    nc.tensor.matmul(
                        avt[0:65, q0:q1], v_sb[jc][r0:r0 + 64, 0:65],
                        ft[r0:r0 + 64, 0:w],
                        start=first, stop=False, skip_group_check=True)

            # ---- far-field + slot0 band, s-outer; v_proj during s=0 ----
            for s in range(4):
                savl = psp.tile([128, 512], dt.float32, name="savl", tag="savl")
                savh = psp.tile([128, 512], dt.float32, name="savh", tag="savh")
                pend_av = []
                for u, (jc, q0, q1) in enumerate(FAR[s]):
                    if s == 0:
                        v_proj(jc)
                    w = q1 - q0
                    if FAR_ALT and u % 2 == 1:
                        kh, qh = kT4[64:128, :], qT4[64:128, :]
                    else:
                        kh, qh = kT_sb[0][0:64, :], qT_sb[0][0:64, :]
                    ps = ring_tile()
                    nc.tensor.matmul(ps[:, 0:w], kh[:, ts(jc, 128)],
                                     qh[:, s * 512 + q0:s * 512 + q1],
                                     start=True, stop=True,
                                     skip_group_check=True)
                    ft = fatp.tile([128, 512], dt.bfloat16, name="fat",
                                   tag="fat", bufs=4)
                    nc.scalar.activation(ft[:, 0:w], ps[:, 0:w], AF.Exp,
                                         bias=fb_t[:, 0:1], scale=float(SCALE))
                    pend_av.append((jc, q0, q1, ft, u == 0))
                    if w == 512:
                        dbg_tiles["ft"] = ft
                    emit_next_proj_chunk()
                    if len(pend_av) > 2:
                        far_av(savl, savh, *pend_av.pop(0))
                while pend_av:
                    far_av(savl, savh, *pend_av.pop(0))
                # slot0's banded part for this s, accumulating into savl/savh
                st0 = band_prep(0, s, use_ring=True)
                for ci in range(3):
                    band_chunk(st0, ci)
                    emit_next_proj_chunk()
                band_mask(st0)
                band_av(st0, savl, savh, cont=True)
                divide(0, s, savl, savh)
                dbg_tiles["at0"] = st0["at"]
                emit_next_proj_chunk()
                emit_next_proj_chunk()

            # ---- banded stages slots 1..7, parity-paired, chunk-interleaved.
            # Retiring stages' AV matmuls (and divides/outproj) are fed into
            # a work queue pumped between chunk emissions so the PE never
            # head-of-line blocks on a psum-bank WAR waiting for an exp.
            pend = []
            avcnt = [0]
            work = []

            def pump(n):
                for _ in range(n):
                    if work:
                        work.pop(0)()

            def outproj_tcn(tcn):
                pss = [psp.tile([128, 512], dt.float32, name=f"ops{oc}",
                                tag=("savl", "savh")[oc]) for oc in range(2)]
                for fc in range(4):
                    for oc in range(2):
                        nc.tensor.matmul(pss[oc][:],
                                         outTn[fc][:, ts(tcn, 128)],
                                         wo_t[fc][:, ts(oc, 512)],
                                         start=(fc == 0), stop=(fc == 3))
                for oc in range(2):
                    ob = att.tile([128, 512], dt.bfloat16, name="ob", tag="ob")
                    nc.vector.tensor_copy(ob[:], pss[oc][:])
                    nc.sync.dma_start(out[ts(tcn, 128), ts(oc, 512)], ob[:])

            def av_piece(st, av, i):
                slot, s, odjs, at = st["slot"], st["s"], st["odjs"], st["at"]
                dj = odjs[i]
                jc = s * 4 - 1 + dj
                c0, c1 = WIN[dj]
                w = c1 - c0
                nc.tensor.matmul(
                    av[0:65, c0:c1],
                    v_sb[jc][:, slot * 72:slot * 72 + 65],
                    at[:, POS[dj]:POS[dj] + w],
                    start=(i == 0), stop=(i == len(odjs) - 1),
                    skip_group_check=True)

            def retire(entry):
                st, av = entry
                for i in range(len(st["djs"])):
                    work.append(lambda st=st, av=av, i=i: av_piece(st, av, i))
                work.append(lambda st=st, av=av:
                            divide(st["slot"], st["s"], av))
                if st["slot"] == 7:
                    for tcn in range(4 * st["s"], 4 * st["s"] + 4):
                        work.append(lambda tcn=tcn: outproj_tcn(tcn))

            def push_stage(st):
                av = psp.tile([128, 512], dt.float32, name="avb",
                              tag=f"rg{avcnt[0] % 2}")
                avcnt[0] += 1
                pend.append((st, av))
                if len(pend) > 2:
                    retire(pend.pop(0))

            for s in range(4):
                for pair in ((1, 2), (3, 4), (5, 6), (7, None)):
                    stA = band_prep(pair[0], s, use_ring=False)
                    stB = band_prep(pair[1], s, use_ring=False) if pair[1] else None
                    for ci in range(3):
                        band_chunk(stA, ci)
                        pump(2)
                        if stB is not None:
                            band_chunk(stB, ci)
                        pump(2)
                    band_mask(stA)
                    if stB is not None:
                        band_mask(stB)
                    push_stage(stA)
                    if stB is not None:
                        push_stage(stB)
                    pump(6)
            while pend:
                retire(pend.pop(0))
            pump(len(work))
            if DEBUG_DUMP:
                nc.sync.dma_start(dbg_avs[:, :], dbg_tiles["avs"][:])
                nc.sync.dma_start(dbg_rec[:, :], dbg_tiles["rec"][:])
                nc.sync.dma_start(dbg_at[:, :], dbg_tiles["at0"][:, 0:1408])
                nc.sync.dma_start(dbg_ft[:, :], dbg_tiles["ft"][:])
                nc.sync.dma_start(dbg_q[:, :], qT_sb[0][:])
                nc.sync.dma_start(dbg_k4[:, :], kT4[64:128, :])
                nc.sync.dma_start(dbg_v[:, :], v_sb[0][:, 0:65])
                for fc in range(4):
                    nc.sync.dma_start(dbg_o[fc * 128:(fc + 1) * 128, :],
                                      outTn[fc][:])
    nc.finalize()
    return nc


def _host_inputs(x, wq, bq, wk, bk, wv, bv, wo, bo):
    """Build the 8 per-core input dicts."""
    r = np.arange(128)[:, None]
    lm = np.zeros((128, 1536), np.float32)
    for dj, ((c0, c1), pos) in enumerate(zip(WIN, POS)):
        c = np.arange(c0, c1)[None, :]
        lm[:, pos:pos + (c1 - c0)] = (np.abs((dj - 1) * 128 + r - c) <= LK)
    lm = _bf16(lm)
    lm_ones = _bf16(np.ones((128, 1536), np.float32))

    fb0 = np.zeros((128, 1), np.float32)
    fb1 = np.full((128, 1), -60.0, np.float32)

    in_maps = []
    for core in range(NCORES):
        b, hg = core // 2, core % 2
        fsl = slice(hg * FG, (hg + 1) * FG)
        in_maps.append({
            "xT": _bf16(x[b].T),
            "wqT": _bf16(wq[fsl].T),
            "wkT": _bf16(wk[fsl].T),
            "wvT": _bf16(wv[fsl].T),
            "woT": _bf16(wo[:, fsl].T),
            "bqc": np.ascontiguousarray(bq[fsl].reshape(4, 128).T, np.float32),
            "lmask": lm,
            "lmask0": lm_ones if hg == 0 else lm,
            "fbias": fb0 if hg == 0 else fb1,
        })
    return in_maps


def kernel(x, wq, bq, wk, bk, wv, bv, wo, bo):
    from concourse.bass_utils import run_bass_kernel_spmd

    x, wq, bq, wk, bk, wv, bv, wo, bo = (
        np.asarray(a, np.float32) for a in (x, wq, bq, wk, bk, wv, bv, wo, bo))

    if "nc" not in _cache:
        _cache["nc"] = _build()
    nc = _cache["nc"]

    in_maps = _host_inputs(x, wq, bq, wk, bk, wv, bv, wo, bo)
    res = run_bass_kernel_spmd(nc, in_maps, core_ids=list(range(NCORES)))
    _cache["last_results"] = res

    const = (bo + bv @ wo.T).astype(np.float32)        # [1024]
    out = np.empty((B, N, E), np.float32)
    for b in range(B):
        out[b] = (np.asarray(res.results[2 * b]["out"], np.float32)
                  + np.asarray(res.results[2 * b + 1]["out"], np.float32)
                  + const)
    return out


# revision 38
# speedup vs baseline: 1.1150x; 1.1150x over previous
"""Local/global multihead attention on 8 NeuronCores (Trainium2, Bass/Tile).

Sharding: core c = b*2 + hg  (b = batch 0..3, hg = head-group 0/1, 8 heads each).
Each core computes q/k/v projections for its 8 heads on its batch, head-local
attention (slot 0 runs a dense 2048-key path driven by a per-core mask so the
SPMD program is uniform: hg0's slot 0 is the true global head with an all-ones
mask, hg1's slot 0 is a local head with a band mask), banded attention with
narrowed tq windows for slots 1-7, and the output projection restricted to its
head-group columns of wo. Host sums the two head-group partials per batch and
adds bo + bv @ wo.T (valid because softmax rows sum to 1; bk is dropped
entirely since exp(q.bk) cancels in softmax).

v2 notes vs v1:
- softmax denominators via vector.reciprocal_approx_fast (custom DVE op,
  ~5x faster than InstReciprocal which measured 4us per [1,512]).
- banded score windows packed into one [128,1536] at tile per (h,s): one
  mask multiply instead of six.
- q bias folded into the scalar-engine PSUM->SBUF copy (activation bias);
  k bias dropped (softmax-invariant).
- q/k projections loop-reordered so 4 matmuls share one LoadStationary.
- out-projection PSUM->SBUF copies moved to gpsimd.
- software pipelining: v-projection chunks and AV matmuls staggered inside
  the dense jc loop; banded (h,s) stages staggered by one.

All matmul operands are bf16 (TensorE runs 1 cyc/row vs 4 for fp32); PSUM
accumulation is fp32 throughout.
"""
import numpy as np
import ml_dtypes

E, H, D, LK = 1024, 16, 64, 128
SCALE = D ** -0.5
B, N = 4, 2048
FG = 512          # features per head-group (8 heads * 64)
NCORES = 8

# narrowed tq windows per dj variant (delta = (dj-1)*128)
WIN = [(0, 128), (0, 256), (0, 384), (128, 512), (256, 512), (384, 512)]
# packed column offsets of each dj window inside the [128,1536] at tile
POS = [0, 128, 384, 768, 1152, 1408]
# psum packing: P0=[dj0,dj1] P1=[dj2] P2=[dj3] P3=[dj4,dj5]
PGRP = [(0, 0), (0, 128), (1, 0), (2, 0), (3, 0), (3, 256)]  # (ptile, col0)

_cache = {}


def _bf16(a):
    return np.ascontiguousarray(a.astype(ml_dtypes.bfloat16))


def _build():
    import concourse.bacc as bacc
    import concourse.tile as tile
    import concourse.mybir as mybir
    from concourse.bass import ts

    dt = mybir.dt
    AF = mybir.ActivationFunctionType

    nc = bacc.Bacc("TRN2", target_bir_lowering=False, debug=False,
                   num_devices=NCORES)

    xT = nc.dram_tensor("xT", [E, N], dt.bfloat16, kind="ExternalInput")
    wqT = nc.dram_tensor("wqT", [E, FG], dt.bfloat16, kind="ExternalInput")
    wkT = nc.dram_tensor("wkT", [E, FG], dt.bfloat16, kind="ExternalInput")
    wvT = nc.dram_tensor("wvT", [E, FG], dt.bfloat16, kind="ExternalInput")
    woT = nc.dram_tensor("woT", [FG, E], dt.bfloat16, kind="ExternalInput")
    bqc = nc.dram_tensor("bqc", [128, 4], dt.float32, kind="ExternalInput")
    lmask = nc.dram_tensor("lmask", [128, 1536], dt.bfloat16, kind="ExternalInput")
    gmask = nc.dram_tensor("gmask", [16, 128, N], dt.bfloat16, kind="ExternalInput")
    out = nc.dram_tensor("out", [N, E], dt.bfloat16, kind="ExternalOutput")

    with tile.TileContext(nc) as tc:
        with (
            tc.tile_pool(name="wts", bufs=1) as wts,
            tc.tile_pool(name="xp", bufs=1) as xp,
            tc.tile_pool(name="qkv", bufs=1) as qkv,
            tc.tile_pool(name="att", bufs=3) as att,
            tc.tile_pool(name="gm", bufs=2) as gm,
            tc.tile_pool(name="small", bufs=4) as small,
            tc.tile_pool(name="ps", bufs=1, space="PSUM") as psp,
        ):
            # ---- load weights/x/masks (ordered so qk_proj(0) starts ASAP) ----
            xT_t = [xp.tile([128, N], dt.bfloat16, name=f"xT{i}", tag=f"xT{i}") for i in range(8)]
            wq_t = [wts.tile([128, FG], dt.bfloat16, name=f"wq{i}", tag=f"wq{i}") for i in range(8)]
            wk_t = [wts.tile([128, FG], dt.bfloat16, name=f"wk{i}", tag=f"wk{i}") for i in range(8)]
            wv_t = [wts.tile([128, FG], dt.bfloat16, name=f"wv{i}", tag=f"wv{i}") for i in range(8)]
            bq_t = small.tile([128, 4], dt.float32, name="bq", tag="bq")
            nc.sync.dma_start(bq_t[:], bqc[:, :])
            for ec in range(8):
                nc.sync.dma_start(xT_t[ec][:], xT[ts(ec, 128), :])
                nc.sync.dma_start(wq_t[ec][:], wqT[ts(ec, 128), :])
                nc.sync.dma_start(wk_t[ec][:], wkT[ts(ec, 128), :])
            for ec in range(8):
                nc.sync.dma_start(wv_t[ec][:], wvT[ts(ec, 128), :])
            lm_t = wts.tile([128, 1536], dt.bfloat16, name="lm", tag="lm")
            nc.sync.dma_start(lm_t[:], lmask[:, :])
            wo_t = [wts.tile([128, E], dt.bfloat16, name=f"wo{i}", tag=f"wo{i}") for i in range(4)]
            for fc in range(4):
                nc.sync.dma_start(wo_t[fc][:], woT[ts(fc, 128), :])

            qT_sb = [qkv.tile([128, N], dt.bfloat16, name=f"qT{i}", tag=f"qT{i}") for i in range(4)]
            kT_sb = [qkv.tile([128, N], dt.bfloat16, name=f"kT{i}", tag=f"kT{i}") for i in range(4)]
            v_sb = [qkv.tile([128, 8 * 72], dt.bfloat16, name=f"v{i}", tag=f"v{i}") for i in range(16)]
            outTn = [qkv.tile([128, N], dt.bfloat16, name=f"outTn{i}", tag=f"outTn{i}") for i in range(4)]

            def qk_proj(fc):
                """project q and k feature chunk fc (128 rows of qT/kT)."""
                for dst, w_t, biased in ((qT_sb, wq_t, True), (kT_sb, wk_t, False)):
                    accs = [psp.tile([128, 512], dt.float32, name=f"acc{t}",
                                     tag=f"acc{t}") for t in range(4)]
                    for ec in range(8):
                        for tcn in range(4):
                            nc.tensor.matmul(
                                accs[tcn][:], w_t[ec][:, ts(fc, 128)],
                                xT_t[ec][:, ts(tcn, 512)],
                                start=(ec == 0), stop=(ec == 7))
                    for tcn in range(4):
                        if biased:
                            nc.scalar.activation(
                                dst[fc][:, ts(tcn, 512)], accs[tcn][:],
                                AF.Identity, bias=bq_t[:, fc:fc + 1])
                        else:
                            nc.vector.tensor_copy(dst[fc][:, ts(tcn, 512)],
                                                  accs[tcn][:])

            def v_proj(tcn):
                """v chunk tcn: natural layout, per-head 72-col strided + ones col."""
                ps = psp.tile([128, 512], dt.float32, name="ps", tag="ps", bufs=2)
                for ec in range(8):
                    nc.tensor.matmul(ps[:], xT_t[ec][:, ts(tcn, 128)], wv_t[ec][:],
                                     start=(ec == 0), stop=(ec == 7))
                src = ps[:].rearrange("p (h d) -> p h d", h=8)
                dst = v_sb[tcn][:].rearrange("p (h d) -> p h d", h=8)[:, :, 0:64]
                nc.vector.tensor_copy(dst, src)
                ones = v_sb[tcn][:].rearrange("p (h d) -> p h d", h=8)[:, :, 64:65]
                nc.vector.memset(ones, 1.0)

            def head_rows(t, h):
                r0 = (h % 2) * 64
                return t[h // 2][r0:r0 + 64, :]

            def divide(h, s, av):
                """outTn rows for (h, s-block) = av numerators / denominator."""
                den = small.tile([1, 512], dt.float32, name="den", tag="den")
                nc.vector.tensor_copy(den[:], av[64:65, :])
                rec = small.tile([1, 512], dt.float32, name="rec", tag="rec")
                nc.vector.reciprocal_approx_fast(out=rec[:], in_=den[:])
                rec64 = small.tile([64, 512], dt.float32, name="rec64", tag="rec64")
                nc.gpsimd.partition_broadcast(rec64[:], rec[:])
                nc.vector.tensor_mul(head_rows(outTn, h)[:, ts(s, 512)],
                                     av[0:64, :], rec64[:])

            # ================== emission ==================
            qk_proj(0)

            # ---- slot 0: dense 2048-key path with gmask, jc-outer.
            # Stage jc emits: v-proj chunk jc, gmask DMA, QK+exp+mask for jc,
            # then the AV for jc-1 (stagger keeps PE busy while scalar works).
            h = 0
            qh = head_rows(qT_sb, h)
            kh = head_rows(kT_sb, h)
            av_g = [psp.tile([128, 512], dt.float32, name=f"avg{t}",
                             tag=f"acc{t}") for t in range(4)]
            g_at = [None] * 16
            for jc in range(16):
                gt = gm.tile([128, N], dt.bfloat16, name="gm", tag="gm")
                nc.sync.dma_start(gt[:], gmask[jc, :, :])
                at = att.tile([128, N], dt.bfloat16, name="gat", tag="gat", bufs=4)
                g_at[jc] = at
                for s in range(4):
                    ps = psp.tile([128, 512], dt.float32, name="ps", tag="ps", bufs=2)
                    nc.tensor.matmul(ps[:], kh[:, ts(jc, 128)], qh[:, ts(s, 512)],
                                     start=True, stop=True)
                    nc.scalar.activation(at[:, ts(s, 512)], ps[:], AF.Exp,
                                         scale=float(SCALE))
                v_proj(jc)
                nc.vector.tensor_mul(at[:], at[:], gt[:])
                if jc > 0:
                    for s in range(4):
                        nc.tensor.matmul(
                            av_g[s][0:65, :], v_sb[jc - 1][:, h * 72:h * 72 + 65],
                            g_at[jc - 1][:, ts(s, 512)], start=(jc - 1 == 0),
                            stop=False, skip_group_check=True)
            for s in range(4):
                nc.tensor.matmul(
                    av_g[s][0:65, :], v_sb[15][:, h * 72:h * 72 + 65],
                    g_at[15][:, ts(s, 512)], start=False, stop=True,
                    skip_group_check=True)
            for s in range(4):
                divide(0, s, av_g[s])

            # ---- slots 1..7: banded path, (h,s) stages staggered by one ----
            stages = []
            for h in range(1, 8):
                for s in range(4):
                    stages.append((h, s))

            pend = []  # [(h, s, av, at, djs)] awaiting AV emission

            def outproj_block(s):
                """output projection for token block s (4 tcn chunks)."""
                for tcn in range(4 * s, 4 * s + 4):
                    t0 = (tcn % 2) * 2
                    pss = [psp.tile([128, 512], dt.float32, name=f"ops{oc}",
                                    tag=f"acc{t0 + oc}") for oc in range(2)]
                    for fc in range(4):
                        for oc in range(2):
                            nc.tensor.matmul(pss[oc][:],
                                             outTn[fc][:, ts(tcn, 128)],
                                             wo_t[fc][:, ts(oc, 512)],
                                             start=(fc == 0), stop=(fc == 3))
                    for oc in range(2):
                        ob = att.tile([128, 512], dt.bfloat16, name="ob", tag="ob")
                        nc.vector.tensor_copy(ob[:], pss[oc][:])
                        nc.sync.dma_start(out[ts(tcn, 128), ts(oc, 512)], ob[:])

            def emit_qk(h, s):
                qh = head_rows(qT_sb, h)
                kh = head_rows(kT_sb, h)
                djs = [dj for dj in range(6) if 0 <= s * 4 - 1 + dj <= 15]
                at = att.tile([128, 1536], dt.bfloat16, name="at", tag="at", bufs=4)
                ptiles = [None] * 4
                for dj in djs:
                    pt, pc0 = PGRP[dj]
                    if ptiles[pt] is None:
                        ptiles[pt] = psp.tile([128, 512], dt.float32, name="bps",
                                              tag="ps", bufs=2)
                    jc = s * 4 - 1 + dj
                    c0, c1 = WIN[dj]
                    w = c1 - c0
                    nc.tensor.matmul(ptiles[pt][:, pc0:pc0 + w], kh[:, ts(jc, 128)],
                                     qh[:, s * 512 + c0:s * 512 + c1],
                                     start=True, stop=True, skip_group_check=True)
                # exps: one per packed psum tile, into packed at positions
                done = set()
                for dj in djs:
                    pt, pc0 = PGRP[dj]
                    if pt in done:
                        continue
                    done.add(pt)
                    # full extent of this ptile used by djs present
                    lo = min(PGRP[d][1] for d in djs if PGRP[d][0] == pt)
                    hi = max(PGRP[d][1] + WIN[d][1] - WIN[d][0]
                             for d in djs if PGRP[d][0] == pt)
                    atlo = min(POS[d] for d in djs if PGRP[d][0] == pt)
                    nc.scalar.activation(at[:, atlo:atlo + (hi - lo)],
                                         ptiles[pt][:, lo:hi], AF.Exp,
                                         scale=float(SCALE))
                # one mask multiply over the packed tile
                lo = min(POS[d] for d in djs)
                hi = max(POS[d] + WIN[d][1] - WIN[d][0] for d in djs)
                nc.vector.tensor_mul(at[:, lo:hi], at[:, lo:hi], lm_t[:, lo:hi])
                av = psp.tile([128, 512], dt.float32, name="av", tag="av", bufs=2)
                return av, at, djs

            def emit_av(h, s, av, at, djs):
                for i, dj in enumerate(djs):
                    jc = s * 4 - 1 + dj
                    c0, c1 = WIN[dj]
                    nc.tensor.matmul(
                        av[0:65, c0:c1], v_sb[jc][:, h * 72:h * 72 + 65],
                        at[:, POS[dj]:POS[dj] + (c1 - c0)],
                        start=(i == 0), stop=(i == len(djs) - 1),
                        skip_group_check=True)

            def retire(entry):
                ph, psn, pav, pat, pdjs = entry
                emit_av(ph, psn, pav, pat, pdjs)
                divide(ph, psn, pav)
                if ph == 7:
                    # head 7 is the last writer of outTn token block psn:
                    # its output projection can stream out now, overlapping
                    # the remaining banded stages
                    outproj_block(psn)

            for h, s in stages:
                if h >= 2 and s == 0 and h % 2 == 0:
                    qk_proj(h // 2)
                av, at, djs = emit_qk(h, s)
                pend.append((h, s, av, at, djs))
                if len(pend) > 1:
                    retire(pend.pop(0))
            while pend:
                retire(pend.pop(0))
    nc.finalize()
    return nc


def _host_inputs(x, wq, bq, wk, bk, wv, bv, wo, bo):
    """Build the 8 per-core input dicts."""
    r = np.arange(128)[:, None]
    lm = np.zeros((128, 1536), np.float32)
    for dj, ((c0, c1), pos) in enumerate(zip(WIN, POS)):
        c = np.arange(c0, c1)[None, :]
        lm[:, pos:pos + (c1 - c0)] = (np.abs((dj - 1) * 128 + r - c) <= LK)
    lm = _bf16(lm)

    cN = np.arange(N)[None, :]
    gm_band = np.zeros((16, 128, N), np.float32)
    for jc in range(16):
        gm_band[jc] = (np.abs(128 * jc + r - cN) <= LK)
    gm_ones = _bf16(np.ones((16, 128, N), np.float32))
    gm_band = _bf16(gm_band)

    in_maps = []
    for core in range(NCORES):
        b, hg = core // 2, core % 2
        fsl = slice(hg * FG, (hg + 1) * FG)
        in_maps.append({
            "xT": _bf16(x[b].T),
            "wqT": _bf16(wq[fsl].T),
            "wkT": _bf16(wk[fsl].T),
            "wvT": _bf16(wv[fsl].T),
            "woT": _bf16(wo[:, fsl].T),
            "bqc": np.ascontiguousarray(bq[fsl].reshape(4, 128).T, np.float32),
            "lmask": lm,
            "gmask": gm_ones if hg == 0 else gm_band,
        })
    return in_maps


def kernel(x, wq, bq, wk, bk, wv, bv, wo, bo):
    from concourse.bass_utils import run_bass_kernel_spmd

    x, wq, bq, wk, bk, wv, bv, wo, bo = (
        np.asarray(a, np.float32) for a in (x, wq, bq, wk, bk, wv, bv, wo, bo))

    if "nc" not in _cache:
        _cache["nc"] = _build()
    nc = _cache["nc"]

    in_maps = _host_inputs(x, wq, bq, wk, bk, wv, bv, wo, bo)
    res = run_bass_kernel_spmd(nc, in_maps, core_ids=list(range(NCORES)))
    _cache["last_results"] = res

    const = (bo + bv @ wo.T).astype(np.float32)        # [1024]
    out = np.empty((B, N, E), np.float32)
    for b in range(B):
        out[b] = (np.asarray(res.results[2 * b]["out"], np.float32)
                  + np.asarray(res.results[2 * b + 1]["out"], np.float32)
                  + const)
    return out

